# revision 1
# baseline (speedup 1.0000x reference)
"""ChebConv (K=5) Trainium2 Bass kernel, 8-core SPMD.

Math (per reference): x0 = x transposed to [V, D]; T_0=x0, T_1=L@x0,
T_k = 2L@T_{k-1} - T_{k-2}; out[b,fout,v,xyz] = sum_{k,fin} T_k[v,(fin,b,xyz)]
* W[k,fin,fout] + bias.

Strategy:
- Shard D = Fin*B*X*Y*Z over 8 cores by (b, x-pair): core i handles
  b = i//4, x in {2*(i%4), 2*(i%4)+1}  ->  per-core D_loc = 32fin * 128xyz.
  Per-core x0 slice layout: [V=768, 4096] with column d = xz*32 + fin.
- The sparse L (768x768, ~6k nnz) is densified on the host; each spmm is a
  dense [768,768] @ [768, chunk] matmul on TensorE in float32r (full-rate
  fp32 mode). Chebyshev recurrence runs on PSUM->SBUF copies with the
  axpy fused into the copy (DVE).
- The final (K*Fin x Fout) GEMM contracts fin, which lives in the free
  axis -> each basis chunk is cast to bf16 (ACT) and PE-transposed to
  [(xz4,fin32), vo] layout, then a block-diagonal W (bf16) accumulates all
  K,fin into PSUM with vo back on partitions.
- Output per core is [V=768, xz*Fout=4096] f32, reassembled on host.
"""

import numpy as np
import ml_dtypes

B, FIN, V, X, Y, Z = 2, 32, 768, 8, 8, 8
K, FOUT = 5, 32
XYZ = X * Y * Z
NCORES = 8
XZL = 128            # xyz positions per core (2 x-planes * 64)
DLOC = XZL * FIN     # 4096 columns per core
VT = V // 128        # 6 v partition tiles
CH = 512             # chunk columns (16 xz * 32 fin)
NCH = DLOC // CH     # 8 chunks
XZC = CH // FIN      # 16 xz per chunk
DB = CH // 128       # 4 d-blocks (of 128) per chunk

_cache = {}


def _build_nc(reps=1, stages=("spmm", "tr", "gemm")):
    import concourse.bass as bass
    import concourse.bacc as bacc
    import concourse.mybir as mybir
    from concourse.tile import TileContext
    import contextlib

    f32 = mybir.dt.float32
    f32r = mybir.dt.float32r
    bf16 = mybir.dt.bfloat16

    nc = bacc.Bacc(None, target_bir_lowering=False)
    xs = nc.declare_dram_parameter("xs", [V, DLOC], f32r, isOutput=False)
    lt2b = nc.declare_dram_parameter("lt2b", [128, VT * VT * 128], f32r, isOutput=False)
    wtb = nc.declare_dram_parameter("wtb", [128, K * 128], bf16, isOutput=False)
    ident = nc.declare_dram_parameter("ident", [128, 128], bf16, isOutput=False)
    biasr = nc.declare_dram_parameter("biasr", [128, CH], f32, isOutput=False)
    outp = nc.declare_dram_parameter("outp", [V, XZL * FOUT], f32, isOutput=True)

    with TileContext(nc) as tc:
        with (
            tc.tile_pool(name="consts", bufs=1) as cpool,
            tc.tile_pool(name="xgen", bufs=1) as xpool,
            tc.tile_pool(name="x0load", bufs=2) as lpool,
            tc.tile_pool(name="xcast", bufs=2) as bpool,
            tc.tile_pool(name="basisT", bufs=2) as tpool,
            tc.tile_pool(name="osb", bufs=2) as opool,
            tc.tile_pool(name="zp", bufs=3, space="PSUM") as zpool,
            tc.tile_pool(name="ptp", bufs=2, space="PSUM") as ptpool,
            tc.tile_pool(name="pop", bufs=2, space="PSUM") as popool,
        ):
            lt2_sb = cpool.tile([128, VT * VT * 128], f32r)
            wt_sb = cpool.tile([128, K * 128], bf16)
            id_sb = cpool.tile([128, 128], bf16)
            bias_sb = cpool.tile([128, CH], f32)
            nc.sync.dma_start(out=lt2_sb[:], in_=lt2b[:])
            nc.sync.dma_start(out=wt_sb[:], in_=wtb[:])
            nc.sync.dma_start(out=id_sb[:], in_=ident[:])
            nc.sync.dma_start(out=bias_sb[:], in_=biasr[:])

            def lt2_blk(vi, vo):
                s = (vi * VT + vo) * 128
                return lt2_sb[:, s:s + 128]

            if reps > 1:
                rep_cm = tc.For_i(
                    0, reps, 1,
                    hint_engines=(mybir.EngineType.PE, mybir.EngineType.DVE,
                                  mybir.EngineType.Activation,
                                  mybir.EngineType.SP))
            else:
                rep_cm = contextlib.nullcontext()
            with rep_cm:
              for c in range(NCH):
                # ---- load x0 chunk ----
                x0 = []
                for vt in range(VT):
                    t = lpool.tile([128, CH], f32r, tag=f"x0_{vt}")
                    nc.sync.dma_start(
                        out=t[:], in_=xs[vt * 128:(vt + 1) * 128, c * CH:(c + 1) * CH])
                    x0.append(t)

                # basis in normal orientation per k; basisT bf16 tiles per k
                basisT = {}

                def cast_and_transpose(k, xk):
                    # cast to bf16 (ACT), then PE-transpose into
                    # [(d%128) part, vo] bf16 tiles, one per d-block.
                    xb = []
                    for vt in range(VT):
                        tb = bpool.tile([128, CH], bf16, tag=f"xb_{vt}")
                        nc.scalar.copy(out=tb[:], in_=xk[vt][:].bitcast(f32))
                        xb.append(tb)
                    tiles = []
                    for j in range(DB):
                        pt = ptpool.tile([128, V], bf16, tag="pt")
                        for vt in range(VT):
                            nc.tensor.transpose(
                                pt[:, vt * 128:(vt + 1) * 128],
                                xb[vt][:, j * 128:(j + 1) * 128],
                                id_sb[:],
                            )
                        st = tpool.tile([128, V], bf16, tag=f"bT_{k}_{j}")
                        nc.vector.tensor_copy(st[:], pt[:])
                        tiles.append(st)
                    basisT[k] = tiles

                cast_and_transpose(0, x0)

                xprev2, xprev1 = None, x0
                for k in range(1, K):
                    xk = []
                    for vt in range(VT):
                        z = zpool.tile([128, CH], f32, tag="z")
                        for vi in range(VT):
                            nc.tensor.matmul(
                                z[:], lt2_blk(vi, vt),
                                xprev1[vi][:],
                                start=(vi == 0), stop=(vi == VT - 1),
                            )
                        t = xpool.tile([128, CH], f32r, tag=f"x{k}_{vt}")
                        if k == 1:
                            # z = 2L x0 ; T_1 = L x0 = z/2
                            nc.vector.tensor_scalar_mul(t[:], z[:], 0.5)
                        else:
                            # T_k = 2L T_{k-1} - T_{k-2}
                            nc.vector.tensor_sub(t[:], z[:], xprev2[vt][:].bitcast(f32))
                        xk.append(t)
                    cast_and_transpose(k, xk)
                    xprev2, xprev1 = xprev1, xk

                # ---- final GEMM: out[vo, (xz,fo)] over (k, fin) ----
                for vt in range(VT):
                    po = popool.tile([128, CH], f32, tag="po")
                    for j in range(DB):
                        for k in range(K):
                            nc.tensor.matmul(
                                po[:, j * 128:(j + 1) * 128],
                                basisT[k][j][:, vt * 128:(vt + 1) * 128],
                                wt_sb[:, k * 128:(k + 1) * 128],
                                start=(k == 0), stop=(k == K - 1),
                            )
                    ot = opool.tile([128, CH], f32, tag=f"o_{vt}")
                    nc.vector.tensor_add(ot[:], po[:], bias_sb[:])
                    nc.sync.dma_start(
                        out=outp[vt * 128:(vt + 1) * 128, c * CH:(c + 1) * CH],
                        in_=ot[:])
    nc.compile()
    return nc


def _host_prep(lap_rows, lap_cols, lap_vals, x, weight, bias):
    """Build per-core input maps."""
    L = np.zeros((V, V), np.float32)
    np.add.at(L, (np.asarray(lap_rows), np.asarray(lap_cols)),
              np.asarray(lap_vals, np.float32))
    LT2 = (2.0 * L).T.astype(np.float32)
    lt2b = np.ascontiguousarray(
        LT2.reshape(VT, 128, VT, 128).transpose(1, 0, 2, 3).reshape(128, -1))

    w = np.asarray(weight, np.float32)
    wt = np.zeros((K, 128, 128), np.float32)
    for xz in range(4):
        wt[:, xz * FIN:(xz + 1) * FIN, xz * FOUT:(xz + 1) * FOUT] = w
    wtb = np.ascontiguousarray(
        wt.transpose(1, 0, 2).reshape(128, K * 128)).astype(ml_dtypes.bfloat16)

    identity = np.eye(128, dtype=ml_dtypes.bfloat16)
    biasr = np.tile(np.asarray(bias, np.float32), (128, XZC)).astype(np.float32)
    assert biasr.shape == (128, CH)

    xf = np.asarray(x, np.float32)
    in_maps = []
    for i in range(NCORES):
        b, xp = i // 4, i % 4
        xsl = xf[b][:, :, 2 * xp:2 * xp + 2]          # [FIN, V, 2, Y, Z]
        xs = np.ascontiguousarray(
            xsl.transpose(1, 2, 3, 4, 0).reshape(V, DLOC))
        in_maps.append({
            "xs": xs, "lt2b": lt2b, "wtb": wtb,
            "ident": identity, "biasr": biasr,
        })
    return in_maps


def kernel(lap_rows, lap_cols, lap_vals, x, weight, bias):
    from concourse.bass_utils import run_bass_kernel_spmd

    if "nc" not in _cache:
        _cache["nc"] = _build_nc()
    nc = _cache["nc"]

    in_maps = _host_prep(lap_rows, lap_cols, lap_vals, x, weight, bias)
    res = run_bass_kernel_spmd(nc, in_maps, core_ids=list(range(NCORES)))

    out = np.empty((B, FOUT, V, X, Y, Z), np.float32)
    for i in range(NCORES):
        b, xp = i // 4, i % 4
        o = res.results[i]["outp"].reshape(V, 2, Y, Z, FOUT)
        out[b, :, :, 2 * xp:2 * xp + 2] = o.transpose(4, 0, 1, 2, 3)
    return out



# revision 9
# speedup vs baseline: 18.9494x; 18.9494x over previous
"""ChebConv (K=5) Trainium2 Bass kernel, 8-core SPMD — monomial form.

Math: out = x0 @ C0 + sum_{j=1..4} L^j @ (x0 @ Cj) + bias, where the
monomial coefficients Cj come from the Chebyshev expansion
(C0=W0-W2+W4, C1=W1-3W3, C2=2W2-8W4, C3=4W3, C4=8W4).  L acts on the
vertex axis and the Cj on fin, so they commute: all GEMMs apply to the
host-pre-transposed x (fin on partitions) and every L-power apply lands
directly in the output PSUM — no on-device transposes at all.

Per-core: D sharded by (b, x-plane-pair): core i handles b=i//4,
x in {2*(i%4), 2*(i%4)+1} -> 128 xyz positions * 32 fin = 4096 columns.

Precision: GEMMs bf16; L^1, L^2 applies bf16; (L^3, L^4) applies ride a
single fp8e4 DoubleRow pair (2x PE rate) with power-of-2 scales
(32*L^3)(c3/32), (128*L^4)(c4/128) folded into host-side constants so
every fp8 operand sits in healthy e4m3 range (sim: rel 4.2e-3, and
9.3e-3 even if HW flushes fp8 subnormals).

Engines: PE does GEMMs + applies (one mixed-dtype PSUM accumulation
group per out tile); ACT evacuates c1/c2 (f32->bf16); DVE evacuates
c3/c4 (f32->fp8) and the final out tiles (PSUM + bias -> f32 SBUF).
"""

import numpy as np
import ml_dtypes

B, FIN, V, X, Y, Z = 2, 32, 768, 8, 8, 8
K, FOUT = 5, 32
XYZ = X * Y * Z
NCORES = 8
XZL = 128            # xyz positions per core (2 x-planes * 64)
DLOC = XZL * FIN     # 4096 columns per core
VT = V // 128        # 6 vertex partition tiles
CH = 512             # out columns per chunk = 4 groups * (4 xz * 32 fout)
NCH = DLOC // CH     # 8 chunks
G = 4                # (xz4, f32) groups per chunk
S3, S4 = 32.0, 128.0  # fp8 balance scales for L^3 / L^4 terms

_cache = {}


def _build_nc(reps=1):
    import concourse.bacc as bacc
    import concourse.mybir as mybir
    from concourse.tile import TileContext
    import contextlib

    f32 = mybir.dt.float32
    bf16 = mybir.dt.bfloat16
    f8 = mybir.dt.float8e4
    DR = mybir.MatmulPerfMode.DoubleRow

    nc = bacc.Bacc(None, target_bir_lowering=False)
    xt_d = nc.declare_dram_parameter("xt", [128, NCH, G, V], bf16,
                                     isOutput=False)
    l1t_d = nc.declare_dram_parameter("l1t", [128, VT * VT * 128], bf16,
                                      isOutput=False)
    l2t_d = nc.declare_dram_parameter("l2t", [128, VT * VT * 128], bf16,
                                      isOutput=False)
    l34_d = nc.declare_dram_parameter("l34", [128, VT * VT, 2, 128], f8,
                                      isOutput=False)
    wg_d = nc.declare_dram_parameter("wg", [128, 5, 128], bf16,
                                     isOutput=False)
    bias_d = nc.declare_dram_parameter("biasr", [128, CH], f32, isOutput=False)
    out_d = nc.declare_dram_parameter("outp", [V, DLOC], f32, isOutput=True)

    with TileContext(nc) as tc:
        with (
            tc.tile_pool(name="consts", bufs=1) as cpool,
            tc.tile_pool(name="xtp", bufs=2) as xtpool,
            tc.tile_pool(name="c12p", bufs=2) as c12pool,
            tc.tile_pool(name="c34p", bufs=2) as c34pool,
            tc.tile_pool(name="outs", bufs=3) as opool,
            tc.tile_pool(name="gp", bufs=1, space="PSUM") as gpool,
            tc.tile_pool(name="pop", bufs=3, space="PSUM") as popool,
        ):
            l1t = cpool.tile([128, VT * VT * 128], bf16)
            l2t = cpool.tile([128, VT * VT * 128], bf16)
            l34 = cpool.tile([128, VT * VT, 2, 128], f8)
            wg = cpool.tile([128, 5, 128], bf16)
            biasr = cpool.tile([128, CH], f32)
            nc.sync.dma_start(out=l1t[:], in_=l1t_d[:])
            nc.sync.dma_start(out=l2t[:], in_=l2t_d[:])
            nc.sync.dma_start(out=l34[:], in_=l34_d[:])
            nc.sync.dma_start(out=wg[:], in_=wg_d[:])
            nc.sync.dma_start(out=biasr[:], in_=bias_d[:])

            def lblk(t, vi, vo):
                s = (vi * VT + vo) * 128
                return t[:, s:s + 128]

            if reps > 1:
                rep_cm = tc.For_i(
                    0, reps, 1,
                    hint_engines=(mybir.EngineType.PE, mybir.EngineType.DVE,
                                  mybir.EngineType.Activation,
                                  mybir.EngineType.SP))
            else:
                rep_cm = contextlib.nullcontext()
            with rep_cm:
              for c in range(NCH):
                # ---- load this chunk's 4 XT group-slices [128, 4, V] ----
                xtc = xtpool.tile([128, G, V], bf16, tag="xtc")
                nc.sync.dma_start(out=xtc[:], in_=xt_d[:, c, :, :])

                # ---- GEMM phase: c_j[vi] for j=1..4 ----
                c12 = []
                c34 = []
                for vi in range(VT):
                    t12 = c12pool.tile([128, 2, CH], bf16, tag=f"c12_{vi}")
                    t34 = c34pool.tile([128, 2, CH], f8, tag=f"c34_{vi}")
                    c12.append(t12)
                    c34.append(t34)
                for vi in range(VT):
                    pms = []
                    for j in (1, 2, 3, 4):
                        pm = gpool.tile([128, CH], f32, tag=f"pm{j}",
                                        name=f"pm{j}")
                        for g in range(G):
                            nc.tensor.matmul(
                                pm[:, g * 128:(g + 1) * 128],
                                xtc[:, g, vi * 128:(vi + 1) * 128],
                                wg[:, j, :],
                                start=(g == 0), stop=(g == G - 1))
                        pms.append(pm)
                    # evac: c1,c2 on ACT (bf16); c3,c4 on DVE (fp8)
                    nc.scalar.copy(c12[vi][:, 0, :], pms[0][:])
                    nc.scalar.copy(c12[vi][:, 1, :], pms[1][:])
                    nc.vector.tensor_copy(c34[vi][:, 0, :], pms[2][:])
                    nc.vector.tensor_copy(c34[vi][:, 1, :], pms[3][:])

                # ---- apply phase: one PSUM group per out tile ----
                for vt in range(VT):
                    po = popool.tile([128, CH], f32, tag="po")
                    for g in range(G):
                        nc.tensor.matmul(
                            po[:, g * 128:(g + 1) * 128],
                            xtc[:, g, vt * 128:(vt + 1) * 128],
                            wg[:, 0, :],
                            start=(g == 0), stop=False)
                    for vi in range(VT):
                        nc.tensor.matmul(po[:], lblk(l1t, vi, vt),
                                         c12[vi][:, 0, :],
                                         start=False, stop=False)
                    for vi in range(VT):
                        nc.tensor.matmul(po[:], lblk(l2t, vi, vt),
                                         c12[vi][:, 1, :],
                                         start=False, stop=False)
                    for vi in range(VT):
                        for h in range(2):
                            nc.tensor.matmul(
                                po[:, h * 256:(h + 1) * 256],
                                l34[:, vi * VT + vt, :, :],
                                c34[vi][:, :, h * 256:(h + 1) * 256],
                                start=False,
                                stop=(vi == VT - 1 and h == 1),
                                perf_mode=DR)
                    ot = opool.tile([128, CH], f32, tag="ot")
                    nc.vector.scalar_tensor_tensor(
                        ot[:], po[:], 1.0, biasr[:],
                        op0=mybir.AluOpType.mult, op1=mybir.AluOpType.add)
                    nc.sync.dma_start(
                        out=out_d[vt * 128:(vt + 1) * 128,
                                  c * CH:(c + 1) * CH],
                        in_=ot[:])
    nc.compile()
    return nc


def _host_prep(lap_rows, lap_cols, lap_vals, x, weight, bias):
    bf = ml_dtypes.bfloat16
    f8 = ml_dtypes.float8_e4m3fn

    L = np.zeros((V, V), np.float64)
    np.add.at(L, (np.asarray(lap_rows), np.asarray(lap_cols)),
              np.asarray(lap_vals, np.float64))
    L2 = L @ L
    L3 = L2 @ L
    L4 = L2 @ L2

    def blocks(M, dt):
        # [128, (vi, vo), 128] with block (vi,vo) = M.T[vi*128:, vo*128:]
        return np.ascontiguousarray(
            M.T.reshape(VT, 128, VT, 128).transpose(1, 0, 2, 3)
            .reshape(128, -1)).astype(dt)

    l1t = blocks(L.astype(np.float32), bf)
    l2t = blocks(L2.astype(np.float32), bf)
    b3 = (S3 * L3).T.reshape(VT, 128, VT, 128).transpose(1, 0, 2, 3)
    b4 = (S4 * L4).T.reshape(VT, 128, VT, 128).transpose(1, 0, 2, 3)
    l34 = np.ascontiguousarray(
        np.stack([b3, b4], axis=3).reshape(128, VT * VT, 2, 128)).astype(f8)

    W = np.asarray(weight, np.float64)
    Cm = [W[0] - W[2] + W[4], W[1] - 3 * W[3], 2 * W[2] - 8 * W[4],
          4 * W[3] / S3, 8 * W[4] / S4]
    wg = np.zeros((5, 128, 128), np.float32)
    for j in range(5):
        for a in range(4):
            wg[j, a * FIN:(a + 1) * FIN, a * FOUT:(a + 1) * FOUT] = Cm[j]
    wg = np.ascontiguousarray(wg.transpose(1, 0, 2)).astype(bf)

    biasr = np.tile(np.asarray(bias, np.float32), (128, CH // FOUT)).astype(
        np.float32)

    xf = np.asarray(x, np.float32)
    in_maps = []
    for i in range(NCORES):
        b, xp = i // 4, i % 4
        xsl = xf[b][:, :, 2 * xp:2 * xp + 2]           # [FIN, V, 2, Y, Z]
        # xt[p=(a4,fin32), g, v] with xz = g*4 + a
        xt = np.ascontiguousarray(
            xsl.reshape(FIN, V, XZL).transpose(2, 0, 1)   # [xz, fin, v]
            .reshape(32, 4, FIN, V).transpose(1, 2, 0, 3)  # [a, fin, g, v]
            .reshape(128, NCH, G, V)).astype(bf)
        in_maps.append({
            "xt": xt, "l1t": l1t, "l2t": l2t, "l34": l34,
            "wg": wg, "biasr": biasr,
        })
    return in_maps


def kernel(lap_rows, lap_cols, lap_vals, x, weight, bias):
    from concourse.bass_utils import run_bass_kernel_spmd

    if "nc" not in _cache:
        _cache["nc"] = _build_nc()
    nc = _cache["nc"]

    in_maps = _host_prep(lap_rows, lap_cols, lap_vals, x, weight, bias)
    res = run_bass_kernel_spmd(nc, in_maps, core_ids=list(range(NCORES)))

    out = np.empty((B, FOUT, V, X, Y, Z), np.float32)
    for i in range(NCORES):
        b, xp = i // 4, i % 4
        o = res.results[i]["outp"].reshape(V, XZL, FOUT)
        o = o.transpose(2, 0, 1).reshape(FOUT, V, 2, Y, Z)
        out[b, :, :, 2 * xp:2 * xp + 2] = o
    return out
